# revision 8
# baseline (speedup 1.0000x reference)
"""Causal single-head attention on 8 Trainium2 NeuronCores.

Problem: x:[4,2048,1024] f32, W_q/W_k/W_v:[1024,1024] f32.
  q,k,v = x@W; scores = q@k^T/sqrt(d) causal-masked; out = softmax(scores)@v.

Sharding: 8 cores = 4 batches x 2 query-shards (SPMD, identical program,
per-core data). Causal load balance: the 16 query blocks (128 rows each) of a
batch are split between its 2 cores as evens/odds of a pairing chosen so both
cores share one uniform per-pair key-block-count profile [4,8,12,16]
(optimal: 40 key-block iterations/core vs 64 dense).

K/V projections are split between the two cores of a batch (each computes its
sequence-half from its half of x^T) and exchanged with pairwise AllGather
collectives through DRAM bounce buffers — removes the main duplicated
compute. The K gather goes first so attention can start while V gathers
behind the Q^T projection.

Layout trick: everything is computed via out = lhsT.T @ rhs with x fed
PRE-TRANSPOSED from the host (xTh = own half of x[b].T), so the kernel needs
no on-chip transposes at all:
  K^T[d,s] = Wk_blk.T @ xTh    (lhsT = Wk block, natural layout)
  Q^T[d,q] = Wq_blk.T @ xTq
  V[s,d]   = xTh_blk.T @ Wv
  S^T[k,q] = K^T_blk.T @ Q^T   (scores transposed: softmax key-dim = partition)
  P^T      = exp(S^T/32) * mask      (multiplicative post-exp causal mask, host data)
  denom[q] = P^T_blk.T @ ones  ([q,1] per-partition layout for free)
  out[q,d] = P^T_blk.T @ V     (PSUM-accumulated over key blocks)
  out     *= 1/denom           (per-partition broadcast)

All matmul inputs bf16 (1 cycle/row, FWL), f32 PSUM accumulation.
PSUM note: matmul start=True clears has_written for the WHOLE bank, so
interleaved accumulation groups never share a PSUM tile.
"""

import numpy as np
import ml_dtypes

import concourse.mybir as mybir
import concourse.tile as tile
from concourse import bacc
from concourse.bass_utils import run_bass_kernel_spmd
from contextlib import ExitStack

P = 128
S = 2048
D = 1024
H = S // 2  # sequence half per core
NIB = D // P  # 8 contraction blocks
NSB = S // P  # 16 key blocks
NHB = H // P  # 8 key blocks per half
NQB = 8  # local query blocks per core
CNT = [4, 8, 12, 16]  # key blocks per query-block pair (uniform across cores)
F_CNT = [2, 6, 10, 14]  # full-width (256q) key blocks per pair
T_CNT = 2  # tail key blocks per pair at width 128 (high half only)
# Shortest pair first: delays the first use of the partner-half K/V blocks
# (kb>=8) to ~20 iterations in, hiding the gather readback tail.
PAIR_ORDER = [0, 1, 2, 3]
G_EVEN = [0, 2, 4, 6, 9, 11, 13, 15]
G_ODD = [1, 3, 5, 7, 8, 10, 12, 14]
BF = mybir.dt.bfloat16
F32 = mybir.dt.float32
SCALE = 1.0 / 32.0  # 1/sqrt(1024)
bf16 = ml_dtypes.bfloat16
PAIRS = [[0, 1], [2, 3], [4, 5], [6, 7]]

_prog_cache = {}


import os as _os

DEFAULT_PARTS = _os.environ.get("KPARTS", "cc4")


def _build_program(reps: int = 1, parts: str | None = None):
    if parts is None:
        parts = DEFAULT_PARTS
    key = (reps, parts)
    if key in _prog_cache:
        return _prog_cache[key]
    nc = bacc.Bacc("TRN2", target_bir_lowering=False, debug=False, num_devices=8)
    xTh = nc.dram_tensor("xTh", [D, H], BF, kind="ExternalInput").ap()
    xTq = nc.dram_tensor("xTq", [D, NQB * P], BF, kind="ExternalInput").ap()
    Wq = nc.dram_tensor("Wq", [D, D], BF, kind="ExternalInput").ap()
    Wk = nc.dram_tensor("Wk", [D, D], BF, kind="ExternalInput").ap()
    Wv = nc.dram_tensor("Wv", [D, D], BF, kind="ExternalInput").ap()
    masks_w = nc.dram_tensor("masks_w", [8, P, 2 * P], BF, kind="ExternalInput").ap()
    masks_t = nc.dram_tensor("masks_t", [8, P, P], BF, kind="ExternalInput").ap()
    O = nc.dram_tensor("O", [NQB * P, D], BF, kind="ExternalOutput").ap()

    with tile.TileContext(nc) as tc:
        for _rep in range(reps):
            _emit_body(nc, tc, xTh, xTq, Wq, Wk, Wv, masks_w, masks_t, O, parts)

    nc.compile()
    _prog_cache[key] = nc
    return nc


def _emit_body(nc, tc, xTh, xTq, Wq, Wk, Wv, masks_w, masks_t, O, parts="all"):
    with ExitStack() as ctx:
        # Persistent SBUF residents
        res = ctx.enter_context(tc.tile_pool(name="res", bufs=1))
        kT = [res.tile([P, S], BF, tag=f"kT{d}", name=f"kT{d}") for d in range(NIB)]
        qT = [res.tile([P, NQB * P], BF, tag=f"qT{d}", name=f"qT{d}") for d in range(NIB)]
        v = [res.tile([P, D], BF, tag=f"v{s}", name=f"v{s}") for s in range(NSB)]
        ones = res.tile([P, 1], BF, tag="ones", name="ones")
        nc.vector.memset(ones[:], 1.0)

        # "ccp": cc4 gather structure + parity-predicated readbacks. Each core
        # reads back ONLY the partner half from the gather output (cond on
        # partition-id parity); its own half either already sits in place
        # (even cores) or is moved from the local staging buffer kvin (odd
        # cores) with no collective dependency. Chunk readbacks are emitted
        # right after each CC so they fire as soon as that chunk lands.
        pred = parts in ("ccp", "ccp6")
        if parts == "ccp":
            parts = "cc4"
        elif parts == "ccp6":
            parts = "cc6"
        dram = ctx.enter_context(tc.tile_pool(name="dram", bufs=1, space="DRAM"))
        # one merged bounce: rows 0:NIB*P = K^T half, NIB*P: = V half
        kvin = dram.tile([(NIB + NHB) * P, H], BF, name="kvin")
        VCH = 4 if parts == "cc6" else 2  # V-gather chunk count
        if parts in ("cc4", "cc6"):
            kouts = [dram.tile([2, NIB * P // 2, H], BF, name=f"kout4_{j}") for j in range(2)]
            vouts = [
                dram.tile([2, NHB * P // VCH, H], BF, name=f"vout4_{j}")
                for j in range(VCH)
            ]
            kout2 = vout2 = kvout = None
        elif parts in ("cc1", "Anocc"):
            kvout = dram.tile([2, (NIB + NHB) * P, H], BF, name="kvout")
            kout2 = vout2 = None
        else:
            kout2 = dram.tile([2, NIB * P, H], BF, name="kout2")
            vout2 = dram.tile([2, NHB * P, H], BF, name="vout2")
            kvout = None
        kin = kvin[0 : NIB * P, :]
        vin = kvin[NIB * P : (NIB + NHB) * P, :]

        KH = NIB * P // 2  # rows per k-gather chunk
        VH = NHB * P // VCH

        if pred:
            pid = nc.sync.partition_id()
            podd = pid & 1
            peven = 1 - podd
        else:
            podd = peven = None

        def _kout(e, r0, r1):
            if parts in ("cc4", "cc6"):
                j, base = (0, 0) if r1 <= KH else (1, KH)
                return kouts[j][e, r0 - base : r1 - base, :]
            if parts in ("cc1", "Anocc"):
                return kvout[e, r0:r1, :]
            return kout2[e, r0:r1, :]

        def _vout(e, r0, r1):
            if parts in ("cc4", "cc6"):
                j = r0 // VH
                base = j * VH
                return vouts[j][e, r0 - base : r1 - base, :]
            if parts in ("cc1", "Anocc"):
                return kvout[e, NIB * P + r0 : NIB * P + r1, :]
            return vout2[e, r0:r1, :]

        # ---------------- Phase A: projections ----------------
        with ExitStack() as actx:
            xp = actx.enter_context(tc.tile_pool(name="xp", bufs=1))
            wp = actx.enter_context(tc.tile_pool(name="wp", bufs=1))
            aps = actx.enter_context(tc.tile_pool(name="aps", bufs=2, space="PSUM"))

            xt = [xp.tile([P, H], BF, tag=f"x{i}", name=f"x{i}") for i in range(NIB)]
            xtq = [xp.tile([P, NQB * P], BF, tag=f"xq{i}", name=f"xq{i}") for i in range(NIB)]
            wk = [wp.tile([P, D], BF, tag=f"wk{i}", name=f"wk{i}") for i in range(NIB)]
            wq = [wp.tile([P, D], BF, tag=f"wq{i}", name=f"wq{i}") for i in range(NIB)]
            wv = [wp.tile([P, D], BF, tag=f"wv{i}", name=f"wv{i}") for i in range(NIB)]
            # DMA order matters: kTh needs xt+wk first; wv next (vh), then q
            for i in range(NIB):
                nc.sync.dma_start(xt[i][:], xTh[i * P : (i + 1) * P, :])
                nc.sync.dma_start(wk[i][:], Wk[i * P : (i + 1) * P, :])
            for i in range(NIB):
                nc.sync.dma_start(wv[i][:], Wv[i * P : (i + 1) * P, :])
            for i in range(NIB):
                nc.sync.dma_start(xtq[i][:], xTq[i * P : (i + 1) * P, :])
                nc.sync.dma_start(wq[i][:], Wq[i * P : (i + 1) * P, :])

            def proj2(dst_slices, lhsT, rhs_pairs):
                """dst_slices[n] [128, 512] = sum_i lhsT[i].T @ rhs_pairs[i][n].

                n-innermost so each loaded lhsT is reused by 2 consecutive
                matmuls (halves exposed weight-load time).
                """
                if parts == "mmW":  # timing-only: fixed stationary operand
                    lhsT = [wk[0][:, 0:P] for _ in range(NIB)]
                pss = [
                    aps.tile([P, 512], F32, tag=f"aps{n}", name=f"aps{n}")
                    for n in range(2)
                ]
                for i in range(NIB):
                    for n in range(2):
                        nc.tensor.matmul(
                            pss[n][:], lhsT[i], rhs_pairs[i][n],
                            start=(i == 0), stop=(i == NIB - 1),
                        )
                for n in range(2):
                    nc.vector.tensor_copy(dst_slices[n], pss[n][:])

            _mm_only = parts in ("mmA", "mmW")

            # K^T own half, staged into kT[d][:, 0:H]
            for d in range(NIB):
                proj2(
                    [kT[d][:, n * 512 : (n + 1) * 512] for n in range(2)],
                    [wk[i][:, d * P : (d + 1) * P] for i in range(NIB)],
                    [
                        [xt[i][:, n * 512 : (n + 1) * 512] for n in range(2)]
                        for i in range(NIB)
                    ],
                )
                if not _mm_only:
                    nc.sync.dma_start(kin[d * P : (d + 1) * P, :], kT[d][:, 0:H])
                    if pred:
                        # odd cores: own (high-key) half belongs at cols H:2H;
                        # move it from the staging buffer — no CC dependency
                        nc.sync.dma_start(
                            kT[d][:, H : 2 * H],
                            kin[d * P : (d + 1) * P, :],
                            cond=podd,
                        )
                if parts in ("cc4", "cc6") and d in (NIB // 2 - 1, NIB - 1):
                    j = d // (NIB // 2)
                    nc.gpsimd.collective_compute(
                        "AllGather",
                        mybir.AluOpType.bypass,
                        replica_groups=PAIRS,
                        ins=[kin[j * KH : (j + 1) * KH, :].opt()],
                        outs=[kouts[j].opt()],
                    )
                    if pred:
                        for dd in range(j * (NIB // 2), (j + 1) * (NIB // 2)):
                            # partner half only: odd needs slot 0 (low keys),
                            # even needs slot 1 (high keys)
                            nc.sync.dma_start(
                                kT[dd][:, 0:H],
                                _kout(0, dd * P, (dd + 1) * P),
                                cond=podd,
                            )
                            nc.sync.dma_start(
                                kT[dd][:, H : 2 * H],
                                _kout(1, dd * P, (dd + 1) * P),
                                cond=peven,
                            )
            if parts == "nocc":
                for e in range(2):
                    nc.sync.dma_start(kout2[e, :, :], kin[:, :])
            elif parts not in ("cc1", "cc4", "cc6", "Anocc", "mmA", "mmW"):
                nc.gpsimd.collective_compute(
                    "AllGather",
                    mybir.AluOpType.bypass,
                    replica_groups=PAIRS,
                    ins=[kin.opt()],
                    outs=[kout2.opt()],
                )

            # V own half, staged into v[0..NHB)
            for s in range(NHB):
                proj2(
                    [v[s][:, n * 512 : (n + 1) * 512] for n in range(2)],
                    [xt[i][:, s * P : (s + 1) * P] for i in range(NIB)],
                    [
                        [wv[i][:, n * 512 : (n + 1) * 512] for n in range(2)]
                        for i in range(NIB)
                    ],
                )
                if not _mm_only:
                    nc.sync.dma_start(vin[s * P : (s + 1) * P, :], v[s][:, 0:D])
                    if pred:
                        # odd cores: own V half belongs at v[8+s]
                        nc.sync.dma_start(
                            v[NHB + s][:],
                            vin[s * P : (s + 1) * P, :],
                            cond=podd,
                        )
                if parts in ("cc4", "cc6") and (s + 1) % (NHB // VCH) == 0:
                    j = s // (NHB // VCH)
                    nc.gpsimd.collective_compute(
                        "AllGather",
                        mybir.AluOpType.bypass,
                        replica_groups=PAIRS,
                        ins=[vin[j * VH : (j + 1) * VH, :].opt()],
                        outs=[vouts[j].opt()],
                    )
                    if pred:
                        nsc = NHB // VCH
                        for ss in range(j * nsc, (j + 1) * nsc):
                            nc.sync.dma_start(
                                v[ss][:],
                                _vout(0, ss * P, (ss + 1) * P),
                                cond=podd,
                            )
                            nc.sync.dma_start(
                                v[NHB + ss][:],
                                _vout(1, ss * P, (ss + 1) * P),
                                cond=peven,
                            )
            if _mm_only or parts in ("cc4", "cc6"):
                pass
            elif parts == "nocc":
                for e in range(2):
                    nc.sync.dma_start(vout2[e, :, :], vin[:, :])
            elif parts == "Anocc":
                for e in range(2):
                    nc.sync.dma_start(kvout[e, :, :], kvin[:, :])
            elif parts == "cc1":
                nc.gpsimd.collective_compute(
                    "AllGather",
                    mybir.AluOpType.bypass,
                    replica_groups=PAIRS,
                    ins=[kvin.opt()],
                    outs=[kvout.opt()],
                )
            else:
                nc.gpsimd.collective_compute(
                    "AllGather",
                    mybir.AluOpType.bypass,
                    replica_groups=PAIRS,
                    ins=[vin.opt()],
                    outs=[vout2.opt()],
                )

            # load back gathered K^T and V, e=0 halves first (attention
            # consumes kb ascending, so all e=0 data is needed before any e=1)
            if not _mm_only and not pred:
                for e in range(2):
                    for d in range(NIB):
                        nc.sync.dma_start(
                            kT[d][:, e * H : (e + 1) * H],
                            _kout(e, d * P, (d + 1) * P),
                        )
                    for s in range(NHB):
                        nc.sync.dma_start(
                            v[e * NHB + s][:],
                            _vout(e, s * P, (s + 1) * P),
                        )

            # Q^T (overlaps the gathers)
            for d in range(NIB):
                proj2(
                    [qT[d][:, n * 512 : (n + 1) * 512] for n in range(2)],
                    [wq[i][:, d * P : (d + 1) * P] for i in range(NIB)],
                    [
                        [xtq[i][:, n * 512 : (n + 1) * 512] for n in range(2)]
                        for i in range(NIB)
                    ],
                )

        if parts in ("mmA", "mmW"):
            with tc.tile_pool(name="ka", bufs=1) as ka:
                coll = ka.tile([P, 4 * (NIB * 4 + NHB * 2)], BF, name="coll")
                col = 0
                for d in range(NIB):
                    for n in range(2):
                        nc.vector.tensor_copy(
                            coll[:, col : col + 4], kT[d][:, n * 512 : n * 512 + 4]
                        )
                        col += 4
                    for n in range(2):
                        nc.vector.tensor_copy(
                            coll[:, col : col + 4], qT[d][:, n * 512 : n * 512 + 4]
                        )
                        col += 4
                for s in range(NHB):
                    for n in range(2):
                        nc.vector.tensor_copy(
                            coll[:, col : col + 4], v[s][:, n * 512 : n * 512 + 4]
                        )
                        col += 4
                nc.sync.dma_start(O[0:P, 0:col], coll[:, 0:col])
            return

        if parts in ("A", "Anocc"):
            with tc.tile_pool(name="ka", bufs=1) as ka:
                coll = ka.tile([P, 4 * (NIB * 6 + NSB * 2)], BF, name="coll")
                col = 0
                for d in range(NIB):
                    for n in range(4):
                        nc.vector.tensor_copy(
                            coll[:, col : col + 4], kT[d][:, n * 512 : n * 512 + 4]
                        )
                        col += 4
                    for n in range(2):
                        nc.vector.tensor_copy(
                            coll[:, col : col + 4], qT[d][:, n * 512 : n * 512 + 4]
                        )
                        col += 4
                for s in range(NSB):
                    for n in range(2):
                        nc.vector.tensor_copy(
                            coll[:, col : col + 4], v[s][:, n * 512 : n * 512 + 4]
                        )
                        col += 4
                nc.sync.dma_start(O[0:P, 0:col], coll[:, 0:col])
            return

        # ---------------- Phase B: attention ----------------
        # Causal-exact: pair p runs F_CNT[p] full-width (256q) key blocks for
        # both halves, then T_CNT tail blocks at width 128 for the high half
        # only. Masks (host data, parity-specific) cover the last 2 wide and
        # both tail blocks. Pairs run longest-first so the final normalize+
        # store tail is the shortest pair.
        mp = ctx.enter_context(tc.tile_pool(name="mp", bufs=1))
        mw_tiles = [mp.tile([P, 2 * P], BF, tag=f"mw{i}", name=f"mw{i}") for i in range(8)]
        mt_tiles = [mp.tile([P, P], BF, tag=f"mt{i}", name=f"mt{i}") for i in range(8)]
        for i in range(8):
            nc.sync.dma_start(mw_tiles[i][:], masks_w[i, :, :])
            nc.sync.dma_start(mt_tiles[i][:], masks_t[i, :, :])

        spool = ctx.enter_context(tc.tile_pool(name="spool", bufs=2, space="PSUM"))
        avpool = ctx.enter_context(tc.tile_pool(name="avpool", bufs=1, space="PSUM"))
        dpool = ctx.enter_context(tc.tile_pool(name="dpool", bufs=1, space="PSUM"))
        pp = ctx.enter_context(tc.tile_pool(name="pp", bufs=3))
        op = ctx.enter_context(tc.tile_pool(name="op", bufs=4))
        rp = ctx.enter_context(tc.tile_pool(name="rp", bufs=4))

        def emit_out(e, p, av, den):
            lj = 2 * p + e
            r = rp.tile([P, 1], F32, tag="r", name="r")
            nc.vector.reciprocal(r[:], den[e][:])
            for n in range(2):
                osb = op.tile([P, 512], BF, tag="osb", name="osb")
                nc.vector.tensor_scalar_mul(osb[:], av[e][n][:], r[:])
                nc.sync.dma_start(
                    O[lj * P : (lj + 1) * P, n * 512 : (n + 1) * 512], osb[:]
                )

        # Software pipeline: the PE is in-order, so iteration i's AV matmuls
        # would stall on its own Exp (ACT) latency. Emit iteration i+1's
        # score matmuls BEFORE iteration i's AV matmuls — the exp of i then
        # overlaps the scores of i+1 and the PE never waits on ACT.
        sched = [(p, kb) for p in PAIR_ORDER for kb in range(F_CNT[p] + T_CNT)]
        state = {}

        def emit_scores(p, kb):
            wide = kb < F_CNT[p]
            width = 2 * P if wide else P
            qoff = p * 2 * P if wide else p * 2 * P + P
            ps_s = spool.tile([P, 2 * P], F32, tag="ps_s", name="ps_s")
            for d in range(NIB):
                nc.tensor.matmul(
                    ps_s[:, 0:width],
                    kT[d][:, kb * P : (kb + 1) * P],
                    qT[d][:, qoff : qoff + width],
                    start=(d == 0),
                    stop=(d == NIB - 1),
                )
            return ps_s

        def emit_av(p, kb, ps_s):
            F = F_CNT[p]
            wide = kb < F
            if kb == 0:
                state[p] = (
                    [
                        [
                            avpool.tile([P, 512], F32, tag=f"av{e}{n}", name=f"av{e}{n}")
                            for n in range(2)
                        ]
                        for e in range(2)
                    ],
                    [
                        dpool.tile([P, 1], F32, tag=f"den{e}", name=f"den{e}")
                        for e in range(2)
                    ],
                )
            av, den = state[p]
            if wide:
                pT = pp.tile([P, 2 * P], BF, tag="pT", name="pT", bufs=8)
                nc.scalar.activation(
                    pT[:], ps_s[:], mybir.ActivationFunctionType.Exp, scale=SCALE
                )
                if kb >= F - 2:
                    mi = p * 2 + kb - (F - 2)
                    pTm = pp.tile([P, 2 * P], BF, tag="pTm", name="pTm", bufs=4)
                    nc.vector.tensor_mul(pTm[:], pT[:], mw_tiles[mi][:])
                    pT = pTm
                first = kb == 0
                for e in range(2):
                    lhs = pT[:, e * P : (e + 1) * P]
                    last = (kb == F - 1) if e == 0 else False
                    for n in range(2):
                        nc.tensor.matmul(
                            av[e][n][:], lhs, v[kb][:, n * 512 : (n + 1) * 512],
                            start=first, stop=last,
                        )
                    nc.tensor.matmul(den[e][:], lhs, ones[:], start=first, stop=last)
                if kb == F - 1:
                    emit_out(0, p, av, den)  # low half done; overlaps tails
            else:
                pT = pp.tile([P, P], BF, tag="pTt", name="pTt", bufs=4)
                nc.scalar.activation(
                    pT[:], ps_s[:, 0:P], mybir.ActivationFunctionType.Exp, scale=SCALE
                )
                j = kb - F
                pTm = pp.tile([P, P], BF, tag="pTmt", name="pTmt", bufs=4)
                nc.vector.tensor_mul(pTm[:], pT[:], mt_tiles[p * 2 + j][:])
                last = j == T_CNT - 1
                for n in range(2):
                    nc.tensor.matmul(
                        av[1][n][:], pTm[:], v[kb][:, n * 512 : (n + 1) * 512],
                        start=False, stop=last,
                    )
                nc.tensor.matmul(den[1][:], pTm[:], ones[:], start=False, stop=last)
                if last:
                    emit_out(1, p, av, den)

        ps_cur = emit_scores(*sched[0])
        for i, (p, kb) in enumerate(sched):
            ps_nxt = emit_scores(*sched[i + 1]) if i + 1 < len(sched) else None
            emit_av(p, kb, ps_cur)
            ps_cur = ps_nxt


def _build_masks(parity: int):
    """bf16 multiplicative masks in S^T layout [k, q].

    masks_w [8,128,256]: last 2 full-width key blocks of each pair (both
    halves). masks_t [8,128,128]: the 2 tail blocks (high half only).
    Block value: 1 where key_global <= query_global else 0 (tri on diag).
    """
    G = G_EVEN if parity == 0 else G_ODD
    mw = np.zeros((8, P, 2 * P), dtype=np.float32)
    mt = np.zeros((8, P, P), dtype=np.float32)
    tri = (np.arange(P)[:, None] <= np.arange(P)[None, :]).astype(np.float32)

    def blkval(kb, g, blk):
        if kb < g:
            blk[:] = 1.0
        elif kb == g:
            blk[:] = tri
        # else stays 0

    for p in range(4):
        for j in range(2):
            kb = F_CNT[p] - 2 + j
            for half in range(2):
                blkval(kb, G[2 * p + half], mw[p * 2 + j][:, half * P : (half + 1) * P])
        for j in range(T_CNT):
            blkval(F_CNT[p] + j, G[2 * p + 1], mt[p * 2 + j])
    return mw.astype(bf16), mt.astype(bf16)


def _make_in_maps(x, W_q, W_k, W_v):
    x = np.asarray(x, dtype=np.float32)
    Wq16 = np.asarray(W_q, dtype=np.float32).astype(bf16)
    Wk16 = np.asarray(W_k, dtype=np.float32).astype(bf16)
    Wv16 = np.asarray(W_v, dtype=np.float32).astype(bf16)
    masks_by_parity = [_build_masks(0), _build_masks(1)]
    qcols = {}
    for e, G in ((0, G_EVEN), (1, G_ODD)):
        qcols[e] = np.concatenate([np.arange(g * P, (g + 1) * P) for g in G])

    in_maps = []
    for c in range(8):
        b, e = c // 2, c % 2
        xTb = x[b].T.astype(bf16)  # [D, S], contiguous via astype copy
        in_maps.append(
            {
                "xTh": np.ascontiguousarray(xTb[:, e * H : (e + 1) * H]),
                "xTq": np.ascontiguousarray(xTb[:, qcols[e]]),
                "Wq": Wq16,
                "Wk": Wk16,
                "Wv": Wv16,
                "masks_w": masks_by_parity[e][0],
                "masks_t": masks_by_parity[e][1],
            }
        )
    return in_maps


def kernel(x, W_q, W_k, W_v):
    x = np.asarray(x, dtype=np.float32)
    nc = _build_program()
    in_maps = _make_in_maps(x, W_q, W_k, W_v)

    # the axon terminal occasionally drops a transient error
    # (UNAVAILABLE / device reset); retry a few times before giving up
    import time as _time

    last_exc = None
    for attempt in range(4):
        try:
            res = run_bass_kernel_spmd(nc, in_maps, core_ids=list(range(8)))
            break
        except Exception as exc:  # noqa: BLE001
            last_exc = exc
            _time.sleep(15 * (attempt + 1))
    else:
        raise last_exc

    out = np.empty((x.shape[0], S, D), dtype=np.float32)
    for c in range(8):
        b, e = c // 2, c % 2
        G = G_EVEN if e == 0 else G_ODD
        Oc = np.asarray(res.results[c]["O"], dtype=np.float32)
        for lj, g in enumerate(G):
            out[b, g * P : (g + 1) * P, :] = Oc[lj * P : (lj + 1) * P, :]
    return out



# revision 11
# speedup vs baseline: 1.0155x; 1.0155x over previous
"""Causal single-head attention on 8 Trainium2 NeuronCores.

Problem: x:[4,2048,1024] f32, W_q/W_k/W_v:[1024,1024] f32.
  q,k,v = x@W; scores = q@k^T/sqrt(d) causal-masked; out = softmax(scores)@v.

Sharding: 8 cores = 4 batches x 2 query-shards (SPMD, identical program,
per-core data). Causal load balance: the 16 query blocks (128 rows each) of a
batch are split between its 2 cores as evens/odds of a pairing chosen so both
cores share one uniform per-pair key-block-count profile [4,8,12,16]
(optimal: 40 key-block iterations/core vs 64 dense).

K/V projections are split between the two cores of a batch (each computes its
sequence-half from its half of x^T) and exchanged with pairwise AllGather
collectives through DRAM bounce buffers — removes the main duplicated
compute. The K gather goes first so attention can start while V gathers
behind the Q^T projection.

Layout trick: everything is computed via out = lhsT.T @ rhs with x fed
PRE-TRANSPOSED from the host (xTh = own half of x[b].T), so the kernel needs
no on-chip transposes at all:
  K^T[d,s] = Wk_blk.T @ xTh    (lhsT = Wk block, natural layout)
  Q^T[d,q] = Wq_blk.T @ xTq
  V[s,d]   = xTh_blk.T @ Wv
  S^T[k,q] = K^T_blk.T @ Q^T   (scores transposed: softmax key-dim = partition)
  P^T      = exp(S^T/32) * mask      (multiplicative post-exp causal mask, host data)
  denom[q] = P^T_blk.T @ ones  ([q,1] per-partition layout for free)
  out[q,d] = P^T_blk.T @ V     (PSUM-accumulated over key blocks)
  out     *= 1/denom           (per-partition broadcast)

All matmul inputs bf16 (1 cycle/row, FWL), f32 PSUM accumulation.
PSUM note: matmul start=True clears has_written for the WHOLE bank, so
interleaved accumulation groups never share a PSUM tile.
"""

import numpy as np
import ml_dtypes

import concourse.mybir as mybir
import concourse.tile as tile
from concourse import bacc
from concourse.bass_utils import run_bass_kernel_spmd
from contextlib import ExitStack

P = 128
S = 2048
D = 1024
H = S // 2  # sequence half per core
NIB = D // P  # 8 contraction blocks
NSB = S // P  # 16 key blocks
NHB = H // P  # 8 key blocks per half
NQB = 8  # local query blocks per core
CNT = [4, 8, 12, 16]  # key blocks per query-block pair (uniform across cores)
F_CNT = [2, 6, 10, 14]  # full-width (256q) key blocks per pair
T_CNT = 2  # tail key blocks per pair at width 128 (high half only)
# Shortest pair first: delays the first use of the partner-half K/V blocks
# (kb>=8) to ~20 iterations in, hiding the gather readback tail.
PAIR_ORDER = [0, 1, 2, 3]
G_EVEN = [0, 2, 4, 6, 9, 11, 13, 15]
G_ODD = [1, 3, 5, 7, 8, 10, 12, 14]
BF = mybir.dt.bfloat16
F32 = mybir.dt.float32
SCALE = 1.0 / 32.0  # 1/sqrt(1024)
bf16 = ml_dtypes.bfloat16
PAIRS = [[0, 1], [2, 3], [4, 5], [6, 7]]

_prog_cache = {}


import os as _os

DEFAULT_PARTS = _os.environ.get("KPARTS", "cc4")


def _build_program(reps: int = 1, parts: str | None = None):
    if parts is None:
        parts = DEFAULT_PARTS
    key = (reps, parts)
    if key in _prog_cache:
        return _prog_cache[key]
    nc = bacc.Bacc("TRN2", target_bir_lowering=False, debug=False, num_devices=8)
    xTh = nc.dram_tensor("xTh", [D, H], BF, kind="ExternalInput").ap()
    xTq = nc.dram_tensor("xTq", [D, NQB * P], BF, kind="ExternalInput").ap()
    Wq = nc.dram_tensor("Wq", [D, D], BF, kind="ExternalInput").ap()
    Wk = nc.dram_tensor("Wk", [D, D], BF, kind="ExternalInput").ap()
    Wv = nc.dram_tensor("Wv", [D, D], BF, kind="ExternalInput").ap()
    masks_w = nc.dram_tensor("masks_w", [8, P, 2 * P], BF, kind="ExternalInput").ap()
    masks_t = nc.dram_tensor("masks_t", [8, P, P], BF, kind="ExternalInput").ap()
    O = nc.dram_tensor("O", [NQB * P, D], BF, kind="ExternalOutput").ap()

    with tile.TileContext(nc) as tc:
        for _rep in range(reps):
            _emit_body(nc, tc, xTh, xTq, Wq, Wk, Wv, masks_w, masks_t, O, parts)

    nc.compile()
    _prog_cache[key] = nc
    return nc


def _emit_body(nc, tc, xTh, xTq, Wq, Wk, Wv, masks_w, masks_t, O, parts="all"):
    with ExitStack() as ctx:
        # Persistent SBUF residents
        res = ctx.enter_context(tc.tile_pool(name="res", bufs=1))
        kT = [res.tile([P, S], BF, tag=f"kT{d}", name=f"kT{d}") for d in range(NIB)]
        qT = [res.tile([P, NQB * P], BF, tag=f"qT{d}", name=f"qT{d}") for d in range(NIB)]
        v = [res.tile([P, D], BF, tag=f"v{s}", name=f"v{s}") for s in range(NSB)]
        ones = res.tile([P, 1], BF, tag="ones", name="ones")
        nc.vector.memset(ones[:], 1.0)

        # "ccp": cc4 gather structure + parity-predicated readbacks. Each core
        # reads back ONLY the partner half from the gather output (cond on
        # partition-id parity); its own half either already sits in place
        # (even cores) or is moved from the local staging buffer kvin (odd
        # cores) with no collective dependency. Chunk readbacks are emitted
        # right after each CC so they fire as soon as that chunk lands.
        pred = parts in ("ccp", "ccp6")
        if parts == "ccp":
            parts = "cc4"
        elif parts == "ccp6":
            parts = "cc6"
        dram = ctx.enter_context(tc.tile_pool(name="dram", bufs=1, space="DRAM"))
        # one merged bounce: rows 0:NIB*P = K^T half, NIB*P: = V half
        kvin = dram.tile([(NIB + NHB) * P, H], BF, name="kvin")
        VCH = 4 if parts == "cc6" else 2  # V-gather chunk count
        if parts in ("cc4", "cc6"):
            kouts = [dram.tile([2, NIB * P // 2, H], BF, name=f"kout4_{j}") for j in range(2)]
            vouts = [
                dram.tile([2, NHB * P // VCH, H], BF, name=f"vout4_{j}")
                for j in range(VCH)
            ]
            kout2 = vout2 = kvout = None
        elif parts in ("cc1", "Anocc"):
            kvout = dram.tile([2, (NIB + NHB) * P, H], BF, name="kvout")
            kout2 = vout2 = None
        else:
            kout2 = dram.tile([2, NIB * P, H], BF, name="kout2")
            vout2 = dram.tile([2, NHB * P, H], BF, name="vout2")
            kvout = None
        kin = kvin[0 : NIB * P, :]
        vin = kvin[NIB * P : (NIB + NHB) * P, :]

        KH = NIB * P // 2  # rows per k-gather chunk
        VH = NHB * P // VCH

        if pred:
            # SP-issued moves and ACT-issued readbacks each need the parity
            # register on their own sequencer
            podd = nc.sync.partition_id() & 1
            podd_a = nc.scalar.partition_id() & 1
            peven_a = 1 - podd_a
        else:
            podd = podd_a = peven_a = None

        def _kout(e, r0, r1):
            if parts in ("cc4", "cc6"):
                j, base = (0, 0) if r1 <= KH else (1, KH)
                return kouts[j][e, r0 - base : r1 - base, :]
            if parts in ("cc1", "Anocc"):
                return kvout[e, r0:r1, :]
            return kout2[e, r0:r1, :]

        def _vout(e, r0, r1):
            if parts in ("cc4", "cc6"):
                j = r0 // VH
                base = j * VH
                return vouts[j][e, r0 - base : r1 - base, :]
            if parts in ("cc1", "Anocc"):
                return kvout[e, NIB * P + r0 : NIB * P + r1, :]
            return vout2[e, r0:r1, :]

        # ---------------- Phase A: projections ----------------
        with ExitStack() as actx:
            xp = actx.enter_context(tc.tile_pool(name="xp", bufs=1))
            wp = actx.enter_context(tc.tile_pool(name="wp", bufs=1))
            aps = actx.enter_context(tc.tile_pool(name="aps", bufs=2, space="PSUM"))

            xt = [xp.tile([P, H], BF, tag=f"x{i}", name=f"x{i}") for i in range(NIB)]
            xtq = [xp.tile([P, NQB * P], BF, tag=f"xq{i}", name=f"xq{i}") for i in range(NIB)]
            wk = [wp.tile([P, D], BF, tag=f"wk{i}", name=f"wk{i}") for i in range(NIB)]
            wq = [wp.tile([P, D], BF, tag=f"wq{i}", name=f"wq{i}") for i in range(NIB)]
            wv = [wp.tile([P, D], BF, tag=f"wv{i}", name=f"wv{i}") for i in range(NIB)]
            # DMA order matters: kTh needs xt+wk first; wv next (vh), then q
            for i in range(NIB):
                nc.sync.dma_start(xt[i][:], xTh[i * P : (i + 1) * P, :])
                nc.sync.dma_start(wk[i][:], Wk[i * P : (i + 1) * P, :])
            for i in range(NIB):
                nc.sync.dma_start(wv[i][:], Wv[i * P : (i + 1) * P, :])
            for i in range(NIB):
                nc.sync.dma_start(xtq[i][:], xTq[i * P : (i + 1) * P, :])
                nc.sync.dma_start(wq[i][:], Wq[i * P : (i + 1) * P, :])

            def proj2(dst_slices, lhsT, rhs_pairs):
                """dst_slices[n] [128, 512] = sum_i lhsT[i].T @ rhs_pairs[i][n].

                n-innermost so each loaded lhsT is reused by 2 consecutive
                matmuls (halves exposed weight-load time).
                """
                if parts == "mmW":  # timing-only: fixed stationary operand
                    lhsT = [wk[0][:, 0:P] for _ in range(NIB)]
                pss = [
                    aps.tile([P, 512], F32, tag=f"aps{n}", name=f"aps{n}")
                    for n in range(2)
                ]
                for i in range(NIB):
                    for n in range(2):
                        nc.tensor.matmul(
                            pss[n][:], lhsT[i], rhs_pairs[i][n],
                            start=(i == 0), stop=(i == NIB - 1),
                        )
                for n in range(2):
                    nc.vector.tensor_copy(dst_slices[n], pss[n][:])

            _mm_only = parts in ("mmA", "mmW")

            # K^T own half, staged into kT[d][:, 0:H]
            for d in range(NIB):
                proj2(
                    [kT[d][:, n * 512 : (n + 1) * 512] for n in range(2)],
                    [wk[i][:, d * P : (d + 1) * P] for i in range(NIB)],
                    [
                        [xt[i][:, n * 512 : (n + 1) * 512] for n in range(2)]
                        for i in range(NIB)
                    ],
                )
                if not _mm_only:
                    nc.sync.dma_start(kin[d * P : (d + 1) * P, :], kT[d][:, 0:H])
                    if pred:
                        # odd cores: own (high-key) half belongs at cols H:2H;
                        # move it from the staging buffer — no CC dependency
                        nc.sync.dma_start(
                            kT[d][:, H : 2 * H],
                            kin[d * P : (d + 1) * P, :],
                            cond=podd,
                        )
                if parts in ("cc4", "cc6") and d in (NIB // 2 - 1, NIB - 1):
                    j = d // (NIB // 2)
                    nc.gpsimd.collective_compute(
                        "AllGather",
                        mybir.AluOpType.bypass,
                        replica_groups=PAIRS,
                        ins=[kin[j * KH : (j + 1) * KH, :].opt()],
                        outs=[kouts[j].opt()],
                    )
                    if pred:
                        for dd in range(j * (NIB // 2), (j + 1) * (NIB // 2)):
                            # partner half only: odd needs slot 0 (low keys),
                            # even needs slot 1 (high keys). ACT HWDGE queue so
                            # these CC-gated waits don't block SP staging DMAs.
                            nc.scalar.dma_start(
                                kT[dd][:, 0:H],
                                _kout(0, dd * P, (dd + 1) * P),
                                cond=podd_a,
                            )
                            nc.scalar.dma_start(
                                kT[dd][:, H : 2 * H],
                                _kout(1, dd * P, (dd + 1) * P),
                                cond=peven_a,
                            )
            if parts == "nocc":
                for e in range(2):
                    nc.sync.dma_start(kout2[e, :, :], kin[:, :])
            elif parts not in ("cc1", "cc4", "cc6", "Anocc", "mmA", "mmW"):
                nc.gpsimd.collective_compute(
                    "AllGather",
                    mybir.AluOpType.bypass,
                    replica_groups=PAIRS,
                    ins=[kin.opt()],
                    outs=[kout2.opt()],
                )

            # V own half, staged into v[0..NHB)
            for s in range(NHB):
                proj2(
                    [v[s][:, n * 512 : (n + 1) * 512] for n in range(2)],
                    [xt[i][:, s * P : (s + 1) * P] for i in range(NIB)],
                    [
                        [wv[i][:, n * 512 : (n + 1) * 512] for n in range(2)]
                        for i in range(NIB)
                    ],
                )
                if not _mm_only:
                    nc.sync.dma_start(vin[s * P : (s + 1) * P, :], v[s][:, 0:D])
                    if pred:
                        # odd cores: own V half belongs at v[8+s]
                        nc.sync.dma_start(
                            v[NHB + s][:],
                            vin[s * P : (s + 1) * P, :],
                            cond=podd,
                        )
                if parts in ("cc4", "cc6") and (s + 1) % (NHB // VCH) == 0:
                    j = s // (NHB // VCH)
                    nc.gpsimd.collective_compute(
                        "AllGather",
                        mybir.AluOpType.bypass,
                        replica_groups=PAIRS,
                        ins=[vin[j * VH : (j + 1) * VH, :].opt()],
                        outs=[vouts[j].opt()],
                    )
                    if pred:
                        nsc = NHB // VCH
                        for ss in range(j * nsc, (j + 1) * nsc):
                            nc.scalar.dma_start(
                                v[ss][:],
                                _vout(0, ss * P, (ss + 1) * P),
                                cond=podd_a,
                            )
                            nc.scalar.dma_start(
                                v[NHB + ss][:],
                                _vout(1, ss * P, (ss + 1) * P),
                                cond=peven_a,
                            )
            if _mm_only or parts in ("cc4", "cc6"):
                pass
            elif parts == "nocc":
                for e in range(2):
                    nc.sync.dma_start(vout2[e, :, :], vin[:, :])
            elif parts == "Anocc":
                for e in range(2):
                    nc.sync.dma_start(kvout[e, :, :], kvin[:, :])
            elif parts == "cc1":
                nc.gpsimd.collective_compute(
                    "AllGather",
                    mybir.AluOpType.bypass,
                    replica_groups=PAIRS,
                    ins=[kvin.opt()],
                    outs=[kvout.opt()],
                )
            else:
                nc.gpsimd.collective_compute(
                    "AllGather",
                    mybir.AluOpType.bypass,
                    replica_groups=PAIRS,
                    ins=[vin.opt()],
                    outs=[vout2.opt()],
                )

            # load back gathered K^T and V, e=0 halves first (attention
            # consumes kb ascending, so all e=0 data is needed before any e=1)
            if not _mm_only and not pred:
                for e in range(2):
                    for d in range(NIB):
                        nc.sync.dma_start(
                            kT[d][:, e * H : (e + 1) * H],
                            _kout(e, d * P, (d + 1) * P),
                        )
                    for s in range(NHB):
                        nc.sync.dma_start(
                            v[e * NHB + s][:],
                            _vout(e, s * P, (s + 1) * P),
                        )

            # Q^T (overlaps the gathers)
            for d in range(NIB):
                proj2(
                    [qT[d][:, n * 512 : (n + 1) * 512] for n in range(2)],
                    [wq[i][:, d * P : (d + 1) * P] for i in range(NIB)],
                    [
                        [xtq[i][:, n * 512 : (n + 1) * 512] for n in range(2)]
                        for i in range(NIB)
                    ],
                )

        if parts in ("mmA", "mmW"):
            with tc.tile_pool(name="ka", bufs=1) as ka:
                coll = ka.tile([P, 4 * (NIB * 4 + NHB * 2)], BF, name="coll")
                col = 0
                for d in range(NIB):
                    for n in range(2):
                        nc.vector.tensor_copy(
                            coll[:, col : col + 4], kT[d][:, n * 512 : n * 512 + 4]
                        )
                        col += 4
                    for n in range(2):
                        nc.vector.tensor_copy(
                            coll[:, col : col + 4], qT[d][:, n * 512 : n * 512 + 4]
                        )
                        col += 4
                for s in range(NHB):
                    for n in range(2):
                        nc.vector.tensor_copy(
                            coll[:, col : col + 4], v[s][:, n * 512 : n * 512 + 4]
                        )
                        col += 4
                nc.sync.dma_start(O[0:P, 0:col], coll[:, 0:col])
            return

        if parts in ("A", "Anocc"):
            with tc.tile_pool(name="ka", bufs=1) as ka:
                coll = ka.tile([P, 4 * (NIB * 6 + NSB * 2)], BF, name="coll")
                col = 0
                for d in range(NIB):
                    for n in range(4):
                        nc.vector.tensor_copy(
                            coll[:, col : col + 4], kT[d][:, n * 512 : n * 512 + 4]
                        )
                        col += 4
                    for n in range(2):
                        nc.vector.tensor_copy(
                            coll[:, col : col + 4], qT[d][:, n * 512 : n * 512 + 4]
                        )
                        col += 4
                for s in range(NSB):
                    for n in range(2):
                        nc.vector.tensor_copy(
                            coll[:, col : col + 4], v[s][:, n * 512 : n * 512 + 4]
                        )
                        col += 4
                nc.sync.dma_start(O[0:P, 0:col], coll[:, 0:col])
            return

        # ---------------- Phase B: attention ----------------
        # Causal-exact: pair p runs F_CNT[p] full-width (256q) key blocks for
        # both halves, then T_CNT tail blocks at width 128 for the high half
        # only. Masks (host data, parity-specific) cover the last 2 wide and
        # both tail blocks. Pairs run longest-first so the final normalize+
        # store tail is the shortest pair.
        mp = ctx.enter_context(tc.tile_pool(name="mp", bufs=1))
        mw_tiles = [mp.tile([P, 2 * P], BF, tag=f"mw{i}", name=f"mw{i}") for i in range(8)]
        mt_tiles = [mp.tile([P, P], BF, tag=f"mt{i}", name=f"mt{i}") for i in range(8)]
        for i in range(8):
            nc.sync.dma_start(mw_tiles[i][:], masks_w[i, :, :])
            nc.sync.dma_start(mt_tiles[i][:], masks_t[i, :, :])

        spool = ctx.enter_context(tc.tile_pool(name="spool", bufs=2, space="PSUM"))
        avpool = ctx.enter_context(tc.tile_pool(name="avpool", bufs=1, space="PSUM"))
        dpool = ctx.enter_context(tc.tile_pool(name="dpool", bufs=1, space="PSUM"))
        pp = ctx.enter_context(tc.tile_pool(name="pp", bufs=3))
        op = ctx.enter_context(tc.tile_pool(name="op", bufs=4))
        rp = ctx.enter_context(tc.tile_pool(name="rp", bufs=4))

        def emit_out(e, p, av, den):
            lj = 2 * p + e
            r = rp.tile([P, 1], F32, tag="r", name="r")
            nc.vector.reciprocal(r[:], den[e][:])
            for n in range(2):
                osb = op.tile([P, 512], BF, tag="osb", name="osb")
                nc.vector.tensor_scalar_mul(osb[:], av[e][n][:], r[:])
                nc.sync.dma_start(
                    O[lj * P : (lj + 1) * P, n * 512 : (n + 1) * 512], osb[:]
                )

        # Software pipeline: the PE is in-order, so iteration i's AV matmuls
        # would stall on its own Exp (ACT) latency. Emit iteration i+1's
        # score matmuls BEFORE iteration i's AV matmuls — the exp of i then
        # overlaps the scores of i+1 and the PE never waits on ACT.
        sched = [(p, kb) for p in PAIR_ORDER for kb in range(F_CNT[p] + T_CNT)]
        state = {}

        def emit_scores(p, kb):
            wide = kb < F_CNT[p]
            width = 2 * P if wide else P
            qoff = p * 2 * P if wide else p * 2 * P + P
            ps_s = spool.tile([P, 2 * P], F32, tag="ps_s", name="ps_s")
            for d in range(NIB):
                nc.tensor.matmul(
                    ps_s[:, 0:width],
                    kT[d][:, kb * P : (kb + 1) * P],
                    qT[d][:, qoff : qoff + width],
                    start=(d == 0),
                    stop=(d == NIB - 1),
                )
            return ps_s

        def emit_av(p, kb, ps_s):
            F = F_CNT[p]
            wide = kb < F
            if kb == 0:
                state[p] = (
                    [
                        [
                            avpool.tile([P, 512], F32, tag=f"av{e}{n}", name=f"av{e}{n}")
                            for n in range(2)
                        ]
                        for e in range(2)
                    ],
                    [
                        dpool.tile([P, 1], F32, tag=f"den{e}", name=f"den{e}")
                        for e in range(2)
                    ],
                )
            av, den = state[p]
            if wide:
                pT = pp.tile([P, 2 * P], BF, tag="pT", name="pT", bufs=8)
                nc.scalar.activation(
                    pT[:], ps_s[:], mybir.ActivationFunctionType.Exp, scale=SCALE
                )
                if kb >= F - 2:
                    mi = p * 2 + kb - (F - 2)
                    pTm = pp.tile([P, 2 * P], BF, tag="pTm", name="pTm", bufs=4)
                    nc.vector.tensor_mul(pTm[:], pT[:], mw_tiles[mi][:])
                    pT = pTm
                first = kb == 0
                for e in range(2):
                    lhs = pT[:, e * P : (e + 1) * P]
                    last = (kb == F - 1) if e == 0 else False
                    for n in range(2):
                        nc.tensor.matmul(
                            av[e][n][:], lhs, v[kb][:, n * 512 : (n + 1) * 512],
                            start=first, stop=last,
                        )
                    nc.tensor.matmul(den[e][:], lhs, ones[:], start=first, stop=last)
                if kb == F - 1:
                    emit_out(0, p, av, den)  # low half done; overlaps tails
            else:
                pT = pp.tile([P, P], BF, tag="pTt", name="pTt", bufs=4)
                nc.scalar.activation(
                    pT[:], ps_s[:, 0:P], mybir.ActivationFunctionType.Exp, scale=SCALE
                )
                j = kb - F
                pTm = pp.tile([P, P], BF, tag="pTmt", name="pTmt", bufs=4)
                nc.vector.tensor_mul(pTm[:], pT[:], mt_tiles[p * 2 + j][:])
                last = j == T_CNT - 1
                for n in range(2):
                    nc.tensor.matmul(
                        av[1][n][:], pTm[:], v[kb][:, n * 512 : (n + 1) * 512],
                        start=False, stop=last,
                    )
                nc.tensor.matmul(den[1][:], pTm[:], ones[:], start=False, stop=last)
                if last:
                    emit_out(1, p, av, den)

        ps_cur = emit_scores(*sched[0])
        for i, (p, kb) in enumerate(sched):
            ps_nxt = emit_scores(*sched[i + 1]) if i + 1 < len(sched) else None
            emit_av(p, kb, ps_cur)
            ps_cur = ps_nxt


def _build_masks(parity: int):
    """bf16 multiplicative masks in S^T layout [k, q].

    masks_w [8,128,256]: last 2 full-width key blocks of each pair (both
    halves). masks_t [8,128,128]: the 2 tail blocks (high half only).
    Block value: 1 where key_global <= query_global else 0 (tri on diag).
    """
    G = G_EVEN if parity == 0 else G_ODD
    mw = np.zeros((8, P, 2 * P), dtype=np.float32)
    mt = np.zeros((8, P, P), dtype=np.float32)
    tri = (np.arange(P)[:, None] <= np.arange(P)[None, :]).astype(np.float32)

    def blkval(kb, g, blk):
        if kb < g:
            blk[:] = 1.0
        elif kb == g:
            blk[:] = tri
        # else stays 0

    for p in range(4):
        for j in range(2):
            kb = F_CNT[p] - 2 + j
            for half in range(2):
                blkval(kb, G[2 * p + half], mw[p * 2 + j][:, half * P : (half + 1) * P])
        for j in range(T_CNT):
            blkval(F_CNT[p] + j, G[2 * p + 1], mt[p * 2 + j])
    return mw.astype(bf16), mt.astype(bf16)


def _make_in_maps(x, W_q, W_k, W_v):
    x = np.asarray(x, dtype=np.float32)
    Wq16 = np.asarray(W_q, dtype=np.float32).astype(bf16)
    Wk16 = np.asarray(W_k, dtype=np.float32).astype(bf16)
    Wv16 = np.asarray(W_v, dtype=np.float32).astype(bf16)
    masks_by_parity = [_build_masks(0), _build_masks(1)]
    qcols = {}
    for e, G in ((0, G_EVEN), (1, G_ODD)):
        qcols[e] = np.concatenate([np.arange(g * P, (g + 1) * P) for g in G])

    in_maps = []
    for c in range(8):
        b, e = c // 2, c % 2
        xTb = x[b].T.astype(bf16)  # [D, S], contiguous via astype copy
        in_maps.append(
            {
                "xTh": np.ascontiguousarray(xTb[:, e * H : (e + 1) * H]),
                "xTq": np.ascontiguousarray(xTb[:, qcols[e]]),
                "Wq": Wq16,
                "Wk": Wk16,
                "Wv": Wv16,
                "masks_w": masks_by_parity[e][0],
                "masks_t": masks_by_parity[e][1],
            }
        )
    return in_maps


def kernel(x, W_q, W_k, W_v):
    x = np.asarray(x, dtype=np.float32)
    nc = _build_program()
    in_maps = _make_in_maps(x, W_q, W_k, W_v)

    # the axon terminal occasionally drops a transient error
    # (UNAVAILABLE / device reset); retry a few times before giving up
    import time as _time

    last_exc = None
    for attempt in range(4):
        try:
            res = run_bass_kernel_spmd(nc, in_maps, core_ids=list(range(8)))
            break
        except Exception as exc:  # noqa: BLE001
            last_exc = exc
            _time.sleep(15 * (attempt + 1))
    else:
        raise last_exc

    out = np.empty((x.shape[0], S, D), dtype=np.float32)
    for c in range(8):
        b, e = c // 2, c % 2
        G = G_EVEN if e == 0 else G_ODD
        Oc = np.asarray(res.results[c]["O"], dtype=np.float32)
        for lj, g in enumerate(G):
            out[b, g * P : (g + 1) * P, :] = Oc[lj * P : (lj + 1) * P, :]
    return out

